# revision 41
# baseline (speedup 1.0000x reference)
"""Masked transformer encoder layer on 8 trn2 NeuronCores.

Sharding: pure data-parallel — batch B=8, one batch element per core, zero
collectives.  Each core runs the full layer on (N=1024, D=1024, H=16, F=4096).

fp8(e4m3) + DoubleRow tensor-engine pipeline (fp32 accumulation everywhere):
  LN0 (token-major, no affine: g0/beta0 folded into Wqkv'/cqk/bproj') ->
  h bf16 -> PE-transpose -> h^T fp8
  v       = h^T.T @ Wv8 via DR pairs (computed first; AV needs it early)
  q^T,k^T = Wqk8 @ h^T via DR pairs, oc order interleaved (q0,k0,q1,k1,...)
            so head 0's S/exp can start immediately
  S^T     = k^T.T @ q^T per head: DR with zero second tile (K=64)
  P^T     = exp(0.125*S^T + key_mask_bias) -> fp8
  out^T   = v_aug.T @ P^T via DR jc-pairs (row 64 = denominator)
  attn^T  = out^T * bcast(32/denom)  -> fp8 (stores attn*32)
  A       = attn^T.T @ Wproj8 via DR; x1 = srcw + (mq/2048)*A
  LN1(tt) pipelined right after proj(tt); affine folded: x_raw kept, g1 into
  W1', beta1 into b1'/b2'; -> x8, x8/16 fp8 planes (transposed)
  z^T     = (w1_hi, w1_res16) DR-pairs @ (x8, x8/16) ; gelu(psum/64 + b1')
  y       = zT.T @ W2 (bf16) ; out = g1*x_raw + y + b2'
Weight scales: W*64 stored fp8; descales folded into psum-copy constants,
exp scale, gelu scale, and mq_proj.
"""

import numpy as np
import ml_dtypes

import concourse.bass as bass
import concourse.tile as tile
from concourse import bacc
from concourse import mybir
from concourse.bass_utils import run_bass_kernel_spmd

B, N, D, H, F = 8, 1024, 1024, 16, 4096
HD = D // H          # 64
P = 128
FC = D // P          # 8 feature chunks of D
TT = N // P          # 8 token tiles
GC = F // P          # 32 chunks of F
VW = 80              # v tile padded width (stride %16==0); cols: 0:64 v, 64 ones
NEG = -1e30
EPS = 1e-5
SW = 64.0            # weight scale (W*64 stored in fp8)
SA = 32.0            # attnT stored as attn*32

f32 = mybir.dt.float32
bf16 = mybir.dt.bfloat16
f8 = mybir.dt.float8e4
AF = mybir.ActivationFunctionType
OP = mybir.AluOpType
DR = mybir.MatmulPerfMode.DoubleRow


def _pair(t, d1lo):
    """AP over tile slice t with an extra k-tile dim [stride, 2] inserted
    after the partition dim."""
    return bass.AP(tensor=t.tensor, offset=t.offset,
                   ap=[t.ap[0], d1lo] + list(t.ap[1:]))


def _layernorm_inplace_stats(nc, pools, x_ap):
    """Return (mean, rstd) APs ([128,1] each) for x_ap [128, 1024] fp32."""
    stats = pools["stats"].tile([P, 2, 6], f32)
    for sg in range(2):
        nc.vector.bn_stats(out=stats[:, sg, :], in_=x_ap[:, sg * 512:(sg + 1) * 512])
    mv = pools["mv"].tile([P, 2], f32)
    nc.vector.bn_aggr(out=mv[:], in_=stats[:])
    nc.scalar.activation(out=mv[:, 1:2], in_=mv[:, 1:2], func=AF.Sqrt,
                         bias=pools["eps"][:], scale=1.0)
    nc.vector.reciprocal(out=mv[:, 1:2], in_=mv[:, 1:2])
    return mv[:, 0:1], mv[:, 1:2]


def build_bass():
    nc = bacc.Bacc("TRN2")

    # ---------------- DRAM I/O ----------------
    src_h = nc.dram_tensor("src", [N, D], f32, kind="ExternalInput")
    kb_h = nc.dram_tensor("kbias", [TT, P], f32, kind="ExternalInput")
    mq_h = nc.dram_tensor("mq", [TT, P], f32, kind="ExternalInput")
    vecs_h = nc.dram_tensor("vecs", [3, D], f32, kind="ExternalInput")   # g1, bproj', b2'
    cqk_h = nc.dram_tensor("cqk", [16, P], f32, kind="ExternalInput")    # Wqk@beta0 per oc
    mq2_h = nc.dram_tensor("mq2", [2, N], bf16, kind="ExternalInput")    # rows: 1-mq, mq
    bpb_h = nc.dram_tensor("bprojbf", [1, D], bf16, kind="ExternalInput")
    b1_h = nc.dram_tensor("b1r", [GC, P], f32, kind="ExternalInput")
    wqkv_h = nc.dram_tensor("wqkvT", [FC, P, 3 * D], f8, kind="ExternalInput")
    wproj_h = nc.dram_tensor("wprojT", [FC, P, D], f8, kind="ExternalInput")
    w1_h = nc.dram_tensor("w1T", [2 * FC, P, F], f8, kind="ExternalInput")
    w2_h = nc.dram_tensor("w2T", [GC, P, D], bf16, kind="ExternalInput")
    out_h = nc.dram_tensor("out", [N, D], f32, kind="ExternalOutput")

    with TileKernel(nc) as tk:
        tk.run(src_h, kb_h, mq_h, vecs_h, cqk_h, mq2_h, bpb_h, b1_h, wqkv_h,
               wproj_h, w1_h, w2_h, out_h)
    nc.compile()
    return nc


class TileKernel:
    def __init__(self, nc):
        self.nc = nc
        self.tc = tile.TileContext(nc)

    def __enter__(self):
        self.tc.__enter__()
        return self

    def __exit__(self, *a):
        return self.tc.__exit__(*a)

    def run(self, src_h, kb_h, mq_h, vecs_h, cqk_h, mq2_h, bpb_h, b1_h,
            wqkv_h, wproj_h, w1_h, w2_h, out_h):
        nc, tc = self.nc, self.tc
        from contextlib import ExitStack

        with ExitStack() as top:
            consts = top.enter_context(tc.tile_pool(name="consts", bufs=1))
            persist = top.enter_context(tc.tile_pool(name="persist", bufs=1))
            xtp = top.enter_context(tc.tile_pool(name="x1nT", bufs=1))
            tmp_pool = top.enter_context(tc.tile_pool(name="tmp", bufs=2))
            stats_pool = top.enter_context(tc.tile_pool(name="stats", bufs=3))
            mv_pool = top.enter_context(tc.tile_pool(name="mv", bufs=4))
            x1nT = xtp.tile([P, 2, FC, N], f8)   # plane 0 = x8, plane 1 = x8/16

            # ---------- constants ----------
            ident = consts.tile([P, P], bf16)
            from concourse.masks import make_identity
            make_identity(nc, ident[:])
            ones_row = consts.tile([1, P], f32)
            nc.vector.memset(ones_row[:], 1.0)
            ones_col = consts.tile([P, 1], f8)
            nc.vector.memset(ones_col[:], 1.0)
            bc_row = consts.tile([1, P], bf16)      # broadcast row valued 1/SA
            nc.vector.memset(bc_row[:], 1.0 / SA)
            eps_sb = consts.tile([P, 1], f32)
            nc.vector.memset(eps_sb[:], EPS)
            pools = {"stats": stats_pool, "mv": mv_pool, "eps": eps_sb}

            src_sb = persist.tile([P, TT, D], f32)   # src -> srcw -> x1 -> x_raw
            for tt in range(TT):
                nc.sync.dma_start(out=src_sb[:, tt, :],
                                  in_=src_h[tt * P:(tt + 1) * P, :])

            bcast = consts.tile([P, 3, D], f32)

            def _bcast_dma(v3):
                bc_src = bass.AP(tensor=vecs_h[0:1, :].tensor, offset=v3 * D,
                                 ap=[[0, P], [1, D]])
                nc.sync.dma_start(out=bcast[:, v3, :], in_=bc_src)

            kb_sb = consts.tile([P, TT], f32)
            nc.sync.dma_start(out=kb_sb[:], in_=kb_h[:, :].rearrange("a p -> p a"))
            cqk_sb = consts.tile([P, 16], f32)
            nc.sync.dma_start(out=cqk_sb[:], in_=cqk_h[:, :].rearrange("a p -> p a"))
            mq_sb = consts.tile([P, TT], f32)
            nc.sync.dma_start(out=mq_sb[:], in_=mq_h[:, :].rearrange("a p -> p a"))
            invmq_sb = consts.tile([P, TT], f32)
            nc.vector.tensor_scalar(out=invmq_sb[:], in0=mq_sb[:], scalar1=-1.0,
                                    scalar2=1.0, op0=OP.mult, op1=OP.add)
            mqp_sb = consts.tile([P, TT], f32)       # mq / (SA*SW) for proj descale
            nc.vector.tensor_scalar(out=mqp_sb[:], in0=mq_sb[:],
                                    scalar1=1.0 / (SA * SW), scalar2=None, op0=OP.mult)
            b1_sb = consts.tile([P, GC], f32)
            g1b, bprojb, b2b = bcast[:, 0], bcast[:, 1], bcast[:, 2]
            mq2_sb = consts.tile([2, N], bf16)     # rows: 1-mq, mq
            wbp2 = consts.tile([2, D], bf16)       # rows: w(=Wproj@u+bproj'), bproj'
            u_sb = consts.tile([P, FC], f8)        # mean_j v_raw * 32, feature-major

            with ExitStack() as attn_scope:
                qkT = attn_scope.enter_context(tc.tile_pool(name="qkT", bufs=1))
                vp = attn_scope.enter_context(tc.tile_pool(name="vsb", bufs=1))

                # chunks 0..7 = q^T, 8..15 = k^T, 16 = zeros (DR zero tile)
                qkT_sb = qkT.tile([P, 17, N], f8)
                nc.vector.memset(qkT_sb[:, 16, :], 0.0)
                v_sb = vp.tile([P, TT, H, VW], f8)
                nc.vector.memset(v_sb[:, :, :, HD:], 0.0)
                nc.vector.memset(v_sb[:, :, :, HD:HD + 1], 1.0)

                atp = attn_scope.enter_context(tc.tile_pool(name="attnT", bufs=1))
                wpp = attn_scope.enter_context(tc.tile_pool(name="wproj", bufs=1))
                attnT_sb = atp.tile([P, FC, N], f8)
                wproj_sb = wpp.tile([P, FC, D], f8)

                # ====== LN0 + transpose, then attention with interleaved
                # ====== qk/v production (PE has slack under the exp stream)
                with ExitStack() as att:
                    htp = att.enter_context(tc.tile_pool(name="hT", bufs=1))
                    wqkp = att.enter_context(tc.tile_pool(name="wqk", bufs=1))
                    wvp = att.enter_context(tc.tile_pool(name="wv", bufs=1))
                    ptp = att.enter_context(tc.tile_pool(name="pt", bufs=3))
                    rdp = att.enter_context(tc.tile_pool(name="rd", bufs=3))
                    dnp = att.enter_context(tc.tile_pool(name="dn", bufs=3))

                    hT_sb = htp.tile([P, FC, N], f8)
                    wqk_sb = wqkp.tile([P, FC, 2 * D], f8)
                    wv_sb = wvp.tile([P, FC, D], f8)

                    with ExitStack() as ln0_scope:
                        hbp = ln0_scope.enter_context(tc.tile_pool(name="hbf", bufs=2))
                        trps = ln0_scope.enter_context(
                            tc.tile_pool(name="trps", bufs=2, space="PSUM"))
                        for tt in range(TT):
                            x = src_sb[:, tt, :]
                            mean, rstd = _layernorm_inplace_stats(nc, pools, x)
                            hbf = hbp.tile([P, D], bf16)
                            nc.vector.tensor_scalar(out=hbf[:], in0=x, scalar1=mean,
                                                    scalar2=rstd, op0=OP.subtract,
                                                    op1=OP.mult)
                            for fq in range(2):
                                ps = trps.tile([P, 4, P], bf16)
                                for fb in range(4):
                                    nc.tensor.transpose(
                                        ps[:, fb, :],
                                        hbf[:, (4 * fq + fb) * P:(4 * fq + fb + 1) * P],
                                        ident[:])
                                nc.scalar.copy(
                                    hT_sb[:, 4 * fq:4 * fq + 4, tt * P:(tt + 1) * P],
                                    ps[:])

                    bigps = att.enter_context(
                        tc.tile_pool(name="bigps", bufs=2, space="PSUM"))
                    prodps = att.enter_context(
                        tc.tile_pool(name="prodps", bufs=1, space="PSUM"))
                    avps = att.enter_context(
                        tc.tile_pool(name="avps", bufs=2, space="PSUM"))
                    for fc in range(FC):
                        nc.sync.dma_start(out=wqk_sb[:, fc, :],
                                          in_=wqkv_h[fc, :, 0:2 * D])
                    nc.sync.dma_start(out=wv_sb[:],
                                      in_=wqkv_h[:, :, 2 * D:3 * D].rearrange("f p o -> p f o"))
                    nc.sync.dma_start(out=wproj_sb[:],
                                      in_=wproj_h[:, :, :].rearrange("f p o -> p f o"))
                    # late-use constants: defer so they don't delay wqk on the
                    # serial DMA queue (these are consumed only after attention)
                    for v3 in (0, 1, 2):
                        _bcast_dma(v3)
                    nc.sync.dma_start(out=b1_sb[:],
                                      in_=b1_h[:, :].rearrange("g p -> p g"))
                    nc.sync.dma_start(out=mq2_sb[:], in_=mq2_h[:, :])
                    nc.sync.dma_start(out=wbp2[1:2, :], in_=bpb_h[:, :])

                    def qk_chunk(oc):
                        ps = prodps.tile([P, 1024], f32, tag="prod", name="ps")
                        for ib in range(2):
                            for fp_ in range(4):
                                nc.tensor.matmul(
                                    ps[:, ib * 512:(ib + 1) * 512],
                                    wqk_sb[:, 2 * fp_:2 * fp_ + 2, oc * P:(oc + 1) * P],
                                    hT_sb[:, 2 * fp_:2 * fp_ + 2, ib * 512:(ib + 1) * 512],
                                    start=(fp_ == 0), stop=(fp_ == 3), perf_mode=DR)
                        nc.vector.tensor_scalar(out=qkT_sb[:, oc, :], in0=ps[:],
                                                scalar1=1.0 / SW,
                                                scalar2=cqk_sb[:, oc:oc + 1],
                                                op0=OP.mult, op1=OP.add)

                    def v_chunk(tt):
                        ps = prodps.tile([P, 1024], f32, tag="prod", name="ps")
                        for vb in range(2):
                            for fp_ in range(4):
                                nc.tensor.matmul(
                                    ps[:, vb * 512:(vb + 1) * 512],
                                    hT_sb[:, 2 * fp_:2 * fp_ + 2, tt * P:(tt + 1) * P],
                                    wv_sb[:, 2 * fp_:2 * fp_ + 2, vb * 512:(vb + 1) * 512],
                                    start=(fp_ == 0), stop=(fp_ == 3), perf_mode=DR)
                        nc.vector.tensor_scalar(
                            out=v_sb[:, tt, :, 0:HD],
                            in0=ps[:].rearrange("p (h c) -> p h c", h=H),
                            scalar1=1.0 / SW, scalar2=None, op0=OP.mult)

                    qk_chunk(0)
                    qk_chunk(8)

                    for h in range(H):
                        hp = (h % 2) * HD
                        fc_h = h // 2
                        kc, qc = 8 + fc_h, fc_h
                        pt = ptp.tile([P, TT, 1024], f8)
                        for jc in range(TT):
                            ps_s = bigps.tile([P, 1024], f32, tag="ps")
                            for ib in range(2):
                                tl = qkT_sb[hp:hp + HD, kc, jc * P:(jc + 1) * P]
                                tr = qkT_sb[hp:hp + HD, qc, ib * 512:(ib + 1) * 512]
                                nc.tensor.matmul(
                                    ps_s[:, ib * 512:(ib + 1) * 512],
                                    _pair(tl, [(16 - kc) * N, 2]),
                                    _pair(tr, [(16 - qc) * N, 2]),
                                    start=True, stop=True, perf_mode=DR)
                            nc.scalar.activation(out=pt[:, jc, :], in_=ps_s[:], func=AF.Exp,
                                                 bias=kb_sb[:, jc:jc + 1], scale=0.125)
                        # fill PE slack under the exp stream with v / qk
                        # production (ALL v chunks must precede AV(h0) below)
                        if h == 0:
                            qk_chunk(1)
                            qk_chunk(9)
                            for tt in range(TT):
                                v_chunk(tt)
                        elif h % 2 == 1 and h <= 11:
                            qk_chunk(8 + (h + 3) // 2)
                        elif h % 2 == 0 and 2 <= h <= 12:
                            qk_chunk(h // 2 + 1)
                        for ib in range(2):
                            isl = slice(ib * 512, (ib + 1) * 512)
                            ps_av = avps.tile([VW, 512], f32, tag="avbc")
                            for jp in range(4):
                                nc.tensor.matmul(
                                    ps_av[:],
                                    v_sb[:, 2 * jp:2 * jp + 2, h, :],
                                    pt[:, 2 * jp:2 * jp + 2, isl],
                                    start=(jp == 0), stop=(jp == 3), perf_mode=DR)
                            dn = dnp.tile([1, 512], bf16)
                            nc.vector.tensor_copy(dn[:], ps_av[HD:HD + 1, :])
                            ps_b = avps.tile([HD, 512], f32, tag="avbc", name="ps_b")
                            nc.tensor.matmul(ps_b[:], bc_row[:, 0:HD], dn[:],
                                             start=True, stop=True)
                            rd = rdp.tile([HD, 512], f32)
                            nc.vector.reciprocal(rd[:], ps_b[:])
                            nc.vector.tensor_tensor(
                                attnT_sb[hp:hp + HD, fc_h, isl],
                                ps_av[0:HD, :], rd[:], OP.mult)

                # ============ u = (mean_j v_raw)*32 ; w = Wproj@u/scales + bproj' ===
                with tc.tile_pool(name="uwps", bufs=2, space="PSUM") as uwps, \
                        tc.tile_pool(name="wrowp", bufs=1) as wrowp:
                    wrow = wrowp.tile([1, D], f32)
                    for fc in range(FC):
                        ps = uwps.tile([P, 512], f32, tag="ups")
                        for hh in range(2):
                            for jc in range(TT):
                                nc.tensor.matmul(ps[hh * HD:(hh + 1) * HD, 0:1],
                                                 v_sb[:, jc, 2 * fc + hh, 0:HD],
                                                 ones_col[:],
                                                 start=(jc == 0), stop=(jc == TT - 1))
                        nc.vector.tensor_scalar(out=u_sb[:, fc:fc + 1], in0=ps[:, 0:1],
                                                scalar1=SA / N, scalar2=None, op0=OP.mult)
                    for ob in range(2):
                        ps = uwps.tile([P, 512], f32, tag="wps")
                        for fc in range(FC):
                            nc.tensor.matmul(ps[0:1, :], u_sb[:, fc:fc + 1],
                                             wproj_sb[:, fc, ob * 512:(ob + 1) * 512],
                                             start=(fc == 0), stop=(fc == FC - 1))
                        t = tmp_pool.tile([1, 512], f32, tag="wrow")
                        nc.vector.tensor_scalar(out=t[:], in0=ps[0:1, :],
                                                scalar1=1.0 / (SA * SW), scalar2=None,
                                                op0=OP.mult)
                        nc.vector.tensor_tensor(wrow[:, ob * 512:(ob + 1) * 512], t[:],
                                                bprojb[0:1, ob * 512:(ob + 1) * 512], OP.add)
                    nc.vector.tensor_copy(wbp2[0:1, :], wrow[:])

                # ====== proj + x1 + LN1 + x8 planes, pipelined per tt ======
                with ExitStack() as projscope:
                    pps = projscope.enter_context(
                        tc.tile_pool(name="pps", bufs=2, space="PSUM"))
                    wps2 = projscope.enter_context(
                        tc.tile_pool(name="wps2", bufs=2, space="PSUM"))
                    trps2 = projscope.enter_context(
                        tc.tile_pool(name="trps2", bufs=2, space="PSUM"))
                    xbp = projscope.enter_context(tc.tile_pool(name="x1nbf", bufs=2))

                    for tt in range(TT):
                        # srcw adj = (1-mq)^T wb + mq^T bproj' via K=2 matmul
                        ps_w = wps2.tile([P, D], f32)
                        for ob in range(2):
                            osl = slice(ob * 512, (ob + 1) * 512)
                            nc.tensor.matmul(ps_w[:, osl],
                                             mq2_sb[:, tt * P:(tt + 1) * P],
                                             wbp2[:, osl], start=True, stop=True)
                        nc.vector.tensor_tensor(src_sb[:, tt, :], src_sb[:, tt, :],
                                                ps_w[:], OP.add)
                        for ob in range(2):
                            osl = slice(ob * 512, (ob + 1) * 512)
                            ps_p = pps.tile([P, 512], f32)
                            for fp_ in range(4):
                                nc.tensor.matmul(
                                    ps_p[:],
                                    attnT_sb[:, 2 * fp_:2 * fp_ + 2, tt * P:(tt + 1) * P],
                                    wproj_sb[:, 2 * fp_:2 * fp_ + 2, osl],
                                    start=(fp_ == 0), stop=(fp_ == 3), perf_mode=DR)
                            t = tmp_pool.tile([P, 512], f32, tag="x1t")
                            nc.vector.tensor_scalar(out=t[:], in0=ps_p[:],
                                                    scalar1=mqp_sb[:, tt:tt + 1],
                                                    scalar2=None, op0=OP.mult)
                            nc.gpsimd.tensor_tensor(src_sb[:, tt, osl],
                                                    src_sb[:, tt, osl], t[:], OP.add)
                        # LN1 (no affine; g1/beta1 folded) in place -> x_raw
                        x = src_sb[:, tt, :]
                        mean, rstd = _layernorm_inplace_stats(nc, pools, x)
                        nc.vector.tensor_scalar(out=x, in0=x, scalar1=mean,
                                                scalar2=rstd, op0=OP.subtract, op1=OP.mult)
                        xbf = xbp.tile([P, D], bf16)
                        nc.scalar.copy(xbf[:], x)
                        for fq in range(2):
                            ps = trps2.tile([P, 4, P], bf16)
                            for fb in range(4):
                                nc.tensor.transpose(
                                    ps[:, fb, :],
                                    xbf[:, (4 * fq + fb) * P:(4 * fq + fb + 1) * P],
                                    ident[:])
                            nc.scalar.copy(
                                x1nT[:, 0, 4 * fq:4 * fq + 4, tt * P:(tt + 1) * P],
                                ps[:])
                        nc.gpsimd.tensor_scalar(
                            out=x1nT[:, 1, :, tt * P:(tt + 1) * P],
                            in0=x1nT[:, 0, :, tt * P:(tt + 1) * P],
                            scalar1=1.0 / 16.0, scalar2=None, op0=OP.mult)

            # ================= FFN =================
            with ExitStack() as ffn:
                ztp = ffn.enter_context(tc.tile_pool(name="zT", bufs=1))
                zT_sb = ztp.tile([P, GC, N], bf16)
                w2p = ffn.enter_context(tc.tile_pool(name="w2p", bufs=2))
                QW = 256
                w2_tiles = [None] * 4

                def w2_fetch(ob):
                    w2q = w2p.tile([P, GC, QW], bf16)
                    nc.sync.dma_start(out=w2q[:],
                                      in_=w2_h[:, :, ob * QW:(ob + 1) * QW].rearrange("g p c -> p g c"))
                    w2_tiles[ob] = w2q

                w2_fetch(0)   # prefetch during FFN1

                f1 = ffn.enter_context(ExitStack())
                w1p = f1.enter_context(tc.tile_pool(name="w1p", bufs=3))
                zps = f1.enter_context(tc.tile_pool(name="zps", bufs=2, space="PSUM"))

                # ---------------- FFN linear1 (DR: w_hi + w_res16) + gelu ----------------
                for gc in range(GC):
                    w1t = w1p.tile([P, 2 * FC, P], f8)
                    nc.sync.dma_start(out=w1t[:],
                                      in_=w1_h[:, :, gc * P:(gc + 1) * P].rearrange("ft p c -> p ft c"))
                    ps = zps.tile([P, 1024], f32)
                    for ib in range(2):
                        for fc in range(FC):
                            nc.tensor.matmul(ps[:, ib * 512:(ib + 1) * 512],
                                             w1t[:, 2 * fc:2 * fc + 2, :],
                                             x1nT[:, :, fc, ib * 512:(ib + 1) * 512],
                                             start=(fc == 0), stop=(fc == FC - 1),
                                             perf_mode=DR)
                    nc.scalar.activation(out=zT_sb[:, gc, :], in_=ps[:], func=AF.Gelu,
                                         bias=b1_sb[:, gc:gc + 1], scale=1.0 / SW)

                # -------- FFN linear2 (bf16), token-major y, fused residual+out --------
                f1.close()
                with ExitStack() as ffn2:
                    yout = ffn2.enter_context(tc.tile_pool(name="yout", bufs=4))
                    ygp = ffn2.enter_context(tc.tile_pool(name="yg", bufs=4))
                    yps = ffn2.enter_context(tc.tile_pool(name="yps", bufs=3, space="PSUM"))

                    for ob in range(4):
                        osl = slice(ob * QW, (ob + 1) * QW)
                        if ob + 1 < 4:
                            w2_fetch(ob + 1)
                        w2q = w2_tiles[ob]
                        for tt in range(TT):
                            ps = yps.tile([P, QW], f32)
                            for gc in range(GC):
                                nc.tensor.matmul(ps[:],
                                                 zT_sb[:, gc, tt * P:(tt + 1) * P],
                                                 w2q[:, gc, :],
                                                 start=(gc == 0), stop=(gc == GC - 1))
                            # out = y + b2' + g1*x_raw
                            t2 = ygp.tile([P, QW], f32)
                            nc.gpsimd.tensor_tensor(t2[:], src_sb[:, tt, osl],
                                                    g1b[:, osl], OP.mult)
                            t = yout.tile([P, QW], f32)
                            nc.vector.tensor_tensor(t[:], ps[:], b2b[:, osl], OP.add)
                            nc.vector.tensor_tensor(t[:], t[:], t2[:], OP.add)
                            nc.sync.dma_start(out=out_h[tt * P:(tt + 1) * P, osl], in_=t[:])


_NC_CACHE = {}


def _get_nc():
    if "nc" not in _NC_CACHE:
        _NC_CACHE["nc"] = build_bass()
    return _NC_CACHE["nc"]


def prep_in_maps(inputs):
    src = np.asarray(inputs["src"], dtype=np.float32)          # [B, N, D]
    mask = np.asarray(inputs["mask"])                          # [B, N] bool
    Wqkv = np.asarray(inputs["Wqkv"], dtype=np.float32)
    Wproj = np.asarray(inputs["Wproj"], dtype=np.float32)
    bproj = np.asarray(inputs["bproj"], dtype=np.float32)
    W1 = np.asarray(inputs["W1"], dtype=np.float32)
    b1 = np.asarray(inputs["b1"], dtype=np.float32)
    W2 = np.asarray(inputs["W2"], dtype=np.float32)
    b2 = np.asarray(inputs["b2"], dtype=np.float32)
    g0 = np.asarray(inputs["g0"], dtype=np.float32)
    beta0 = np.asarray(inputs["beta0"], dtype=np.float32)
    g1 = np.asarray(inputs["g1"], dtype=np.float32)
    beta1 = np.asarray(inputs["beta1"], dtype=np.float32)

    bf = ml_dtypes.bfloat16
    e4 = ml_dtypes.float8_e4m3

    # fold LN affines into weights/biases (exact host-side math)
    Wqkv_g = Wqkv * g0[None, :]               # qkv = n@Wqkv_g.T + Wqkv@beta0
    cqkv = Wqkv @ beta0                       # [3D]
    cqk = cqkv[0:2 * D]                       # per-feature bias for q,k
    cv = cqkv[2 * D:3 * D]                    # v bias -> folds into bproj'
    bproj2 = bproj + Wproj @ cv
    W1_g = W1 * g1[None, :]
    b1p = b1 + W1 @ beta1
    b2p = b2 + beta1

    wqkvT = np.ascontiguousarray((Wqkv_g.T * SW).reshape(FC, P, 3 * D)).astype(e4)
    wprojT = np.ascontiguousarray((Wproj.T * SW).reshape(FC, P, D)).astype(e4)
    w1s = np.ascontiguousarray(W1_g.T * SW).reshape(FC, P, F)
    w1_hi = w1s.astype(e4)
    w1_res16 = ((w1s - w1_hi.astype(np.float32)) * 16.0).astype(e4)
    w1T = np.ascontiguousarray(
        np.stack([w1_hi, w1_res16], axis=1).reshape(2 * FC, P, F))  # [2FC, P, F]
    w2T = np.ascontiguousarray(W2.T).reshape(GC, P, D).astype(bf)
    vecs = np.ascontiguousarray(np.stack([g1, bproj2, b2p]))
    cqkr = np.ascontiguousarray(cqk.reshape(16, P))
    b1r = np.ascontiguousarray(b1p.reshape(GC, P))
    kbias = np.where(mask, 0.0, NEG).astype(np.float32).reshape(B, TT, P)
    mqf = mask.astype(np.float32).reshape(B, TT, P)
    mq2 = np.stack([1.0 - mask.astype(np.float32), mask.astype(np.float32)],
                   axis=1).astype(bf)  # [B, 2, N]
    bpbf = np.ascontiguousarray(bproj2.reshape(1, D)).astype(bf)

    in_maps = []
    for b in range(B):
        in_maps.append({
            "src": np.ascontiguousarray(src[b]),
            "kbias": np.ascontiguousarray(kbias[b]),
            "mq": np.ascontiguousarray(mqf[b]),
            "vecs": vecs,
            "cqk": cqkr,
            "mq2": np.ascontiguousarray(mq2[b]),
            "bprojbf": bpbf,
            "b1r": b1r,
            "wqkvT": wqkvT,
            "wprojT": wprojT,
            "w1T": w1T,
            "w2T": w2T,
        })
    return in_maps


def kernel(**inputs):
    in_maps = prep_in_maps(inputs)
    nc = _get_nc()
    res = run_bass_kernel_spmd(nc, in_maps, core_ids=list(range(B)))
    return np.stack([r["out"] for r in res.results]).astype(np.float32)
